# revision 1
# baseline (speedup 1.0000x reference)
"""MetaQDA forward on 8 Trainium2 NeuronCores.

Math: the per-class covariance is sigma_c = coef * (B + U_c J U_c^T) with
B = L L^T + kap m^T m shared across classes, U_c = [Xg_c^T, mu_c] (D x 17),
J = diag(1,...,1, -(kap+S)).  Woodbury + the matrix determinant lemma turn
the C=64 dense 512x512 inversions / logdets into rank-17 corrections, and
the Mahalanobis logits become one dense [Q,512] x [512,2752] GEMM plus a
small elementwise epilogue.  Queries are sharded across the 8 cores; the
class statistics (tiny after the reduction) are replicated.
"""
import math
from contextlib import ExitStack

import numpy as np

import concourse.bass as bass
import concourse.tile as tile
from concourse import bacc, mybir
from concourse.bass_utils import run_bass_kernel_spmd

REG = 0.1
D = 512
C = 64
Q = 2048
N_CORES = 8
QC = Q // N_CORES          # 256 queries per core
P = 128                    # partitions
R = None                   # rank per class (S+1), set in prep
F32 = mybir.dt.float32


# ---------------------------------------------------------------- host prep
def _prep(X_support, labels, X_query, m, kappa, nu, triu_diag, triu_lower,
          n_classes):
    f = np.float64
    Xs = np.asarray(X_support, f)
    Nn, Dd = Xs.shape
    Cc = int(n_classes)
    S = Nn // Cc
    r = S + 1
    m_ = np.asarray(m, f).reshape(1, Dd)
    kap = abs(float(kappa)) + 1e-6
    nu_ = max(float(nu), Dd - 1 + 1e-6)

    order = np.argsort(np.asarray(labels), kind="stable")
    Xg = Xs[order].reshape(Cc, S, Dd)
    mu = (kap / (kap + S)) * m_ + (S / (kap + S)) * Xg.mean(axis=1)  # [C,D]

    Lmask = np.tril(np.ones((Dd, Dd), f), -1)
    L = np.diag(np.abs(np.asarray(triu_diag, f))) + np.asarray(triu_lower, f) * Lmask
    B = L @ L.T + kap * (m_.T @ m_)
    coef = (kap + S + 1.0) / ((nu_ + S - Dd + 1.0) * (kap + S))
    alpha = (1.0 - REG) / coef
    common = nu_ + S + 1.0 - Dd
    beta = 0.5 * (common + Dd)

    Binv = np.linalg.inv(B)
    _, ldB = np.linalg.slogdet(B)

    U = np.concatenate([Xg.transpose(0, 2, 1), mu[:, :, None]], axis=2)  # [C,D,r]
    V = np.matmul(Binv, U)                                   # [C,D,r]
    Jinv = np.diag(np.concatenate([np.ones(S), [-1.0 / (kap + S)]]))
    M = Jinv[None] + np.swapaxes(U, 1, 2) @ V                # [C,r,r]
    Ninv = np.linalg.inv(M)
    _, ldM = np.linalg.slogdet(M)

    muB = mu @ Binv                                          # [C,D]
    b = np.einsum("cdr,cd->cr", V, mu)                       # [C,r]
    kq = np.einsum("cd,cd->c", mu, muB)
    VN = V @ Ninv                                            # [C,D,r]
    VNb = np.einsum("cdr,cr->cd", VN, b)
    Nb = np.einsum("crs,cs->cr", Ninv, b)

    linW = (-2.0 * alpha * (muB - VNb) - 2.0 * REG * mu).T   # [D,C]
    cc = (alpha * (kq - np.einsum("cr,cr->c", b, Nb))
          + REG * np.einsum("cd,cd->c", mu, mu) + common)    # [C]

    logdet = Dd * np.log(coef) + ldB + np.log(kap + S) + ldM
    bias = (math.lgamma(0.5 * (common + Dd)) - math.lgamma(0.5 * common)
            - 0.5 * Dd * np.log(common) - 0.5 * logdet)
    gam = bias + beta * np.log(common)                       # [C]

    V_all = V.transpose(1, 0, 2).reshape(Dd, Cc * r)
    E_all = (-alpha * VN).transpose(1, 0, 2).reshape(Dd, Cc * r)
    Wcat = np.concatenate([V_all, E_all, linW, Binv], axis=1)  # [D, 2*C*r+C+D]
    return (Wcat.astype(np.float32), cc.astype(np.float32),
            gam.astype(np.float32), float(alpha), float(beta), r)


# ---------------------------------------------------------------- device IR
_CACHE = {}


def _build(alpha, beta, r):
    NW = 2 * C * r + C + D       # 2752 wcat columns
    WX = QC + NW                 # xqt cols then wcat cols, fused
    nc = bacc.Bacc("TRN2", target_bir_lowering=False, debug=False,
                   num_devices=N_CORES)
    F32R = mybir.dt.float32r
    xq = nc.declare_dram_parameter("xq", [QC, D], F32, isOutput=False)
    wx = nc.declare_dram_parameter("wx", [D, WX], F32R, isOutput=False)
    ccg = nc.declare_dram_parameter("ccg", [P, C], F32, isOutput=False)
    gam = nc.declare_dram_parameter("gam", [P, C], F32, isOutput=False)
    out = nc.declare_dram_parameter("out", [QC, C], F32, isOutput=True)

    KT = D // P                  # 4 k-steps
    QT = QC // P                 # 2 query tiles
    chunks = []
    n0 = 0
    while n0 < NW:
        nw = min(512, NW - n0)
        chunks.append((n0, nw))
        n0 += nw

    wv = wx[:].rearrange("(k p) n -> k p n", p=P)
    xv = xq[:].rearrange("(t p) d -> t p d", p=P)
    ov = out[:].rearrange("(t p) c -> t p c", p=P)

    with tile.TileContext(nc) as tc, ExitStack() as ctx:
        wpool = ctx.enter_context(tc.tile_pool(name="w", bufs=1))
        iopool = ctx.enter_context(tc.tile_pool(name="io", bufs=1))
        opool = ctx.enter_context(tc.tile_pool(name="o", bufs=2))
        spool = ctx.enter_context(tc.tile_pool(name="s", bufs=2))
        pspool = ctx.enter_context(
            tc.tile_pool(name="ps", bufs=4, space="PSUM"))

        w_sb = []
        for k in range(KT):
            wt = wpool.tile([P, WX], F32R, tag=f"w{k}")
            nc.sync.dma_start(wt[:], wv[k])
            w_sb.append(wt)
        cc_sb = iopool.tile([P, C], F32, tag="cc")
        nc.sync.dma_start(cc_sb[:], ccg[:])
        gm_sb = iopool.tile([P, C], F32, tag="gm")
        nc.sync.dma_start(gm_sb[:], gam[:])

        for t in range(QT):
            xq_sb = spool.tile([P, D], F32, tag="xq")
            nc.sync.dma_start(xq_sb[:], xv[t])

            osb = opool.tile([P, NW], F32, tag="osb")
            for (n0, nw) in chunks:
                ps = pspool.tile([P, nw], F32, tag="ps")
                for k in range(KT):
                    nc.tensor.matmul(
                        ps[:], w_sb[k][:, t * P:(t + 1) * P],
                        w_sb[k][:, QC + n0:QC + n0 + nw],
                        start=(k == 0), stop=(k == KT - 1))
                nc.vector.tensor_copy(osb[:, n0:n0 + nw], ps[:])

            # acc = alpha * x^T Binv x + REG * x^T x        [P,1]
            scr = spool.tile([P, D], F32, tag="scr")
            s2 = spool.tile([P, 1], F32, tag="s2")
            nc.scalar.activation(
                scr[:], xq_sb[:], mybir.ActivationFunctionType.Square,
                scale=float(math.sqrt(REG)), accum_out=s2[:])
            scr2 = spool.tile([P, D], F32, tag="scr2")
            g0 = spool.tile([P, 1], F32, tag="g0")
            nc.vector.tensor_mul(scr2[:], osb[:, 2 * C * r + C:NW], xq_sb[:])
            nc.vector.tensor_reduce(
                out=g0[:], in_=scr2[:], axis=mybir.AxisListType.X,
                op=mybir.AluOpType.add)
            acc = spool.tile([P, 1], F32, tag="acc")
            nc.vector.tensor_scalar(
                out=acc[:], in0=g0[:], scalar1=alpha, scalar2=s2[:],
                op0=mybir.AluOpType.mult, op1=mybir.AluOpType.add)

            # seg[q,c] = sum_i A1[q,c,i] * A2[q,c,i]  (the -alpha x^T VNV^T x term)
            prod = spool.tile([P, C * r], F32, tag="prod")
            nc.vector.tensor_mul(prod[:], osb[:, 0:C * r], osb[:, C * r:2 * C * r])
            seg = spool.tile([P, C], F32, tag="seg")
            nc.vector.tensor_reduce(
                out=seg[:], in_=prod[:].rearrange("p (c r) -> p c r", r=r),
                axis=mybir.AxisListType.X, op=mybir.AluOpType.add)

            # tdist = common + dist; logits = gam - beta * ln(tdist)
            td = spool.tile([P, C], F32, tag="td")
            nc.vector.tensor_add(td[:], seg[:], cc_sb[:])
            nc.vector.tensor_add(td[:], td[:], osb[:, 2 * C * r:2 * C * r + C])
            nc.vector.tensor_scalar_add(td[:], td[:], acc[:])
            lg = spool.tile([P, C], F32, tag="lg")
            nc.scalar.activation(lg[:], td[:], mybir.ActivationFunctionType.Ln)
            res = spool.tile([P, C], F32, tag="res")
            nc.vector.tensor_scalar_mul(res[:], lg[:], -beta)
            nc.vector.tensor_add(res[:], res[:], gm_sb[:])
            nc.sync.dma_start(ov[t], res[:])

    nc.compile()
    return nc


def _get_nc(alpha, beta, r):
    key = (round(alpha, 9), round(beta, 9), r)
    if key not in _CACHE:
        _CACHE.clear()
        _CACHE[key] = _build(alpha, beta, r)
    return _CACHE[key]


def kernel(X_support, labels, X_query, m, kappa, nu, triu_diag, triu_lower,
           n_classes):
    Wcat, cc, gam, alpha, beta, r = _prep(
        X_support, labels, X_query, m, kappa, nu, triu_diag, triu_lower,
        n_classes)
    ccg = np.ascontiguousarray(np.broadcast_to(cc[None, :], (P, C)))
    gamg = np.ascontiguousarray(np.broadcast_to(gam[None, :], (P, C)))

    nc = _get_nc(alpha, beta, r)

    Xq = np.asarray(X_query, np.float32)
    in_maps = []
    for i in range(N_CORES):
        sl = np.ascontiguousarray(Xq[i * QC:(i + 1) * QC])
        wxc = np.concatenate([sl.T, Wcat], axis=1)
        in_maps.append({
            "xq": sl,
            "wx": np.ascontiguousarray(wxc),
            "ccg": ccg,
            "gam": gamg,
        })
    res = run_bass_kernel_spmd(nc, in_maps, list(range(N_CORES)))
    return np.concatenate([res.results[i]["out"] for i in range(N_CORES)],
                          axis=0)



# revision 2
# speedup vs baseline: 1.7827x; 1.7827x over previous
"""MetaQDA forward on 8 Trainium2 NeuronCores.

Math: sigma_c = coef * (B + U_c J U_c^T) with B = L L^T + kap m^T m shared,
U_c = [Xg_c^T, mu_c] (D x 17).  Woodbury gives
  sigma_inv_reg_c = K - F_c diag(s) F_c^T,   K = alpha*Binv + REG*I,
and per class the rank-r correction is eigen-factored (QR of V_c = Binv U_c,
then eigh of R Ninv R^T) so a single matrix of <=17 orthogonal columns per
class replaces the V / Ninv V pair.  The Mahalanobis logits then need one
dense fp16 GEMM  xq^T @ [Fpos | Fneg | linW]  plus a tiny fp32 epilogue
(square, segmented reduce, ln).  The shared quadratic x^T K x is computed on
device via a Cholesky block when K is dense; when K is exactly diagonal
(L = I, m = 0) it reduces to a host-side O(Q*D) row-sum shipped as one
scalar per query.  Queries are sharded across the 8 cores; class statistics
(tiny after the reduction) are replicated.
"""
import math
from contextlib import ExitStack

import numpy as np

import concourse.bass as bass
import concourse.tile as tile
from concourse import bacc, mybir
from concourse.bass_utils import run_bass_kernel_spmd

REG = 0.1
D = 512
C = 64
Q = 2048
N_CORES = 8
QC = Q // N_CORES          # 256 queries per core
P = 128                    # partitions
KT = D // P                # 4 k-steps
QT = QC // P               # 2 query tiles
F32 = mybir.dt.float32
F16 = mybir.dt.float16


# ---------------------------------------------------------------- host prep
def _prep(X_support, labels, X_query, m, kappa, nu, triu_diag, triu_lower,
          n_classes):
    f = np.float64
    Xs = np.asarray(X_support, f)
    Nn, Dd = Xs.shape
    Cc = int(n_classes)
    S = Nn // Cc
    r = S + 1
    m_ = np.asarray(m, f).reshape(1, Dd)
    kap = abs(float(kappa)) + 1e-6
    nu_ = max(float(nu), Dd - 1 + 1e-6)

    order = np.argsort(np.asarray(labels), kind="stable")
    Xg = Xs[order].reshape(Cc, S, Dd)
    mu = (kap / (kap + S)) * m_ + (S / (kap + S)) * Xg.mean(axis=1)  # [C,D]

    Lmask = np.tril(np.ones((Dd, Dd), f), -1)
    L = np.diag(np.abs(np.asarray(triu_diag, f))) + np.asarray(triu_lower, f) * Lmask
    B = L @ L.T + kap * (m_.T @ m_)
    coef = (kap + S + 1.0) / ((nu_ + S - Dd + 1.0) * (kap + S))
    alpha = (1.0 - REG) / coef
    common = nu_ + S + 1.0 - Dd
    beta = 0.5 * (common + Dd)

    Binv = np.linalg.inv(B)
    _, ldB = np.linalg.slogdet(B)

    U = np.concatenate([Xg.transpose(0, 2, 1), mu[:, :, None]], axis=2)  # [C,D,r]
    V = np.matmul(Binv, U)                                   # [C,D,r]
    Jinv = np.diag(np.concatenate([np.ones(S), [-1.0 / (kap + S)]]))
    M = Jinv[None] + np.swapaxes(U, 1, 2) @ V                # [C,r,r]
    Ninv = np.linalg.inv(M)
    _, ldM = np.linalg.slogdet(M)

    muB = mu @ Binv
    b = np.einsum("cdr,cd->cr", V, mu)
    kq = np.einsum("cd,cd->c", mu, muB)
    Nb = np.einsum("crs,cs->cr", Ninv, b)
    VNb = np.einsum("cdr,cr->cd", V @ Ninv, b)

    linW = (-2.0 * alpha * (muB - VNb) - 2.0 * REG * mu).T   # [D,C]
    cc = (alpha * (kq - np.einsum("cr,cr->c", b, Nb))
          + REG * np.einsum("cd,cd->c", mu, mu) + common)    # [C]

    logdet = Dd * np.log(coef) + ldB + np.log(kap + S) + ldM
    bias = (math.lgamma(0.5 * (common + Dd)) - math.lgamma(0.5 * common)
            - 0.5 * Dd * np.log(common) - 0.5 * logdet)
    gam = bias + beta * np.log(common)                       # [C]

    # eigen-factor the per-class correction: A_c = V Ninv V^T = P diag(th) P^T
    EPS = 1e-10
    pos_cols = []
    Fneg = np.zeros((Dd, Cc))
    npos = []
    for c in range(Cc):
        Qc, Rc = np.linalg.qr(V[c])
        H = Rc @ Ninv[c] @ Rc.T
        H = 0.5 * (H + H.T)
        th, W = np.linalg.eigh(H)
        Pc = Qc @ W
        keep = np.abs(th) > EPS * np.abs(th).max()
        pos = [Pc[:, i] * math.sqrt(alpha * th[i])
               for i in range(r) if keep[i] and th[i] > 0]
        neg = [Pc[:, i] * math.sqrt(-alpha * th[i])
               for i in range(r) if keep[i] and th[i] < 0]
        assert len(neg) <= 1
        npos.append(len(pos))
        pos_cols.append(pos)
        if neg:
            Fneg[:, c] = neg[0]
    rp = max(npos)
    Fpos = np.zeros((Dd, Cc * rp))
    for c in range(Cc):
        for j, col in enumerate(pos_cols[c]):
            Fpos[:, c * rp + j] = col
    has_neg = bool(np.abs(Fneg).max() > 0)

    K = alpha * Binv + REG * np.eye(Dd)
    kd = np.diag(K).copy()
    fast = bool(np.abs(K - np.diag(kd)).max() < 1e-9 * np.abs(kd).max())

    blocks = []
    if not fast:
        blocks.append(np.linalg.cholesky(K))                 # [D,D]
    blocks.append(Fpos)
    if has_neg:
        blocks.append(Fneg)
    blocks.append(linW)
    W16 = np.ascontiguousarray(np.concatenate(blocks, axis=1).astype(np.float16))

    auxbase = np.empty((P, 2 * C), np.float32)
    auxbase[:, 0:C] = cc.astype(np.float32)[None, :]
    auxbase[:, C:2 * C] = gam.astype(np.float32)[None, :]

    return W16, auxbase, (kd if fast else None), rp, has_neg, fast, float(beta)


# ---------------------------------------------------------------- device IR
_CACHE = {}


def _layout(rp, has_neg, fast):
    """Weight-column layout (after the QC xqt cols) and <=512-col chunks."""
    regions = []
    o = 0
    if not fast:
        regions.append(("R", o, D)); o += D
    regions.append(("F", o, C * rp)); o += C * rp
    if has_neg:
        regions.append(("N", o, C)); o += C
    regions.append(("L", o, C)); o += C
    nw = o
    # chunk boundaries: R is always its own chunk; F split at 512; tail
    # (F remainder + N + L) merged while <= 512.
    cuts = [0]
    for name, start, size in regions:
        if name == "R":
            cuts.append(start + size)
        elif name == "F":
            x = start
            while start + size - x > 512:
                x += 512
                cuts.append(x)
    if nw - cuts[-1] > 512:
        cuts.append(cuts[-1] + 512)
    cuts.append(nw)
    chunks = [(cuts[i], cuts[i + 1] - cuts[i]) for i in range(len(cuts) - 1)
              if cuts[i + 1] > cuts[i]]
    assert len(chunks) <= 4, chunks
    return regions, chunks, nw


def _build(rp, has_neg, fast, beta):
    regions, chunks, NW = _layout(rp, has_neg, fast)
    NWX = QC + NW
    AUXW = 2 * C + (QT if fast else 0)
    nc = bacc.Bacc("TRN2", target_bir_lowering=False, debug=False,
                   num_devices=N_CORES)
    wx = nc.declare_dram_parameter("wx", [D, NWX], F16, isOutput=False)
    aux = nc.declare_dram_parameter("aux", [P, AUXW], F32, isOutput=False)
    out = nc.declare_dram_parameter("out", [QC, C], F32, isOutput=True)

    wxa = wx[:]
    ov = out[:].rearrange("(t p) c -> t p c", p=P)
    rsplit = (512 % rp == 0)

    def overlaps(c0, csz):
        """yield (name, local_off, global_region_off, n) for chunk cols."""
        for name, start, size in regions:
            lo = max(c0, start)
            hi = min(c0 + csz, start + size)
            if hi > lo:
                yield name, lo - c0, lo - start, hi - lo

    with tile.TileContext(nc) as tc, ExitStack() as ctx:
        wpool = ctx.enter_context(tc.tile_pool(name="w", bufs=1))
        iopool = ctx.enter_context(tc.tile_pool(name="io", bufs=1))
        spool = ctx.enter_context(tc.tile_pool(name="s", bufs=2))
        pspool = ctx.enter_context(
            tc.tile_pool(name="ps", bufs=2, space="PSUM"))

        # preload both activation tables while DMA streams in
        junk = iopool.tile([P, 1], F32, tag="junk")
        nc.vector.memset(junk[:], 1.0)
        junk2 = iopool.tile([P, 1], F32, tag="junk2")
        nc.scalar.activation(junk2[:], junk[:],
                             mybir.ActivationFunctionType.Square)
        nc.scalar.activation(junk2[:], junk[:],
                             mybir.ActivationFunctionType.Ln)

        wq = wpool.tile([P, KT, QC], F16, tag="wq")
        nc.sync.dma_start(
            wq[:], wxa[:, 0:QC].rearrange("(k p) n -> p k n", p=P))
        wch = []
        for ci, (c0, csz) in enumerate(chunks):
            wt = wpool.tile([P, KT, csz], F16, tag=f"w{ci}")
            nc.sync.dma_start(
                wt[:], wxa[:, QC + c0:QC + c0 + csz]
                .rearrange("(k p) n -> p k n", p=P))
            wch.append(wt)
        aux_sb = iopool.tile([P, AUXW], F32, tag="aux")
        nc.sync.dma_start(aux_sb[:], aux[:])

        for t in range(QT):
            sq = spool.tile([P, C * rp], F32, tag="sq")
            seg = spool.tile([P, C], F32, tag="seg")
            t1a = spool.tile([P, C], F32, tag="t1a")
            if has_neg:
                sqn = spool.tile([P, C], F32, tag="sqn")
            if not fast:
                scrR = spool.tile([P, D], F32, tag="scrR")
                qsum = spool.tile([P, 1], F32, tag="qsum")
            fdone = 0
            rdone = 0
            for ci, (c0, csz) in enumerate(chunks):
                ps = pspool.tile([P, csz], F32, tag=f"ps{ci}")
                for k in range(KT):
                    nc.tensor.matmul(
                        ps[:], wq[:, k, t * P:(t + 1) * P], wch[ci][:, k, :],
                        start=(k == 0), stop=(k == KT - 1))
                for name, lo, go, n in overlaps(c0, csz):
                    if name == "R":
                        nc.scalar.activation(
                            scrR[:], ps[:],
                            mybir.ActivationFunctionType.Square,
                            accum_out=qsum[:])
                    elif name == "F":
                        if n >= 256:
                            nc.scalar.activation(
                                sq[:, go:go + n], ps[:, lo:lo + n],
                                mybir.ActivationFunctionType.Square)
                        else:
                            nc.vector.tensor_mul(
                                sq[:, go:go + n], ps[:, lo:lo + n],
                                ps[:, lo:lo + n])
                        fdone = go + n
                        if rsplit and fdone % rp == 0:
                            cls0, cls1 = rdone // rp, fdone // rp
                            nc.vector.tensor_reduce(
                                out=seg[:, cls0:cls1],
                                in_=sq[:, rdone:fdone].rearrange(
                                    "p (c r) -> p c r", r=rp),
                                axis=mybir.AxisListType.X,
                                op=mybir.AluOpType.add)
                            rdone = fdone
                    elif name == "N":
                        nc.vector.tensor_mul(
                            sqn[:], ps[:, lo:lo + C], ps[:, lo:lo + C])
                    else:  # L
                        nc.vector.tensor_add(
                            t1a[:], ps[:, lo:lo + C], aux_sb[:, 0:C])
            if rdone < C * rp:
                nc.vector.tensor_reduce(
                    out=seg[:, rdone // rp:C],
                    in_=sq[:, rdone:].rearrange("p (c r) -> p c r", r=rp),
                    axis=mybir.AxisListType.X, op=mybir.AluOpType.add)
            t2a = spool.tile([P, C], F32, tag="t2a")
            nc.vector.tensor_sub(t2a[:], t1a[:], seg[:])
            if has_neg:
                nc.vector.tensor_add(t2a[:], t2a[:], sqn[:])
            lg = spool.tile([P, C], F32, tag="lg")
            qs_ap = aux_sb[:, 2 * C + t:2 * C + t + 1] if fast else qsum[:]
            nc.scalar.activation(lg[:], t2a[:],
                                 mybir.ActivationFunctionType.Ln,
                                 bias=qs_ap, scale=1.0)
            res = spool.tile([P, C], F32, tag="res")
            nc.vector.tensor_scalar_mul(res[:], lg[:], -beta)
            nc.vector.tensor_add(res[:], res[:], aux_sb[:, C:2 * C])
            nc.sync.dma_start(ov[t], res[:])

    nc.compile()
    return nc


def _get_nc(rp, has_neg, fast, beta):
    key = (rp, has_neg, fast, round(beta, 9))
    if key not in _CACHE:
        _CACHE.clear()
        _CACHE[key] = _build(rp, has_neg, fast, beta)
    return _CACHE[key]


def _make_in_maps(inputs):
    W16, auxbase, kd, rp, has_neg, fast, beta = _prep(**inputs)
    nc = _get_nc(rp, has_neg, fast, beta)
    Xq = np.asarray(inputs["X_query"], np.float64)
    AUXW = 2 * C + (QT if fast else 0)
    if fast:
        qs_all = ((Xq * Xq) @ kd).astype(np.float32)         # [Q]
    Xq16 = Xq.astype(np.float16)
    in_maps = []
    for i in range(N_CORES):
        sl = Xq16[i * QC:(i + 1) * QC]
        wxc = np.ascontiguousarray(
            np.concatenate([sl.T, W16], axis=1))
        auxc = np.empty((P, AUXW), np.float32)
        auxc[:, :2 * C] = auxbase
        if fast:
            qs = qs_all[i * QC:(i + 1) * QC]
            for t in range(QT):
                auxc[:, 2 * C + t] = qs[t * P:(t + 1) * P]
        in_maps.append({"wx": wxc, "aux": auxc})
    return nc, in_maps


def kernel(X_support, labels, X_query, m, kappa, nu, triu_diag, triu_lower,
           n_classes):
    nc, in_maps = _make_in_maps(dict(
        X_support=X_support, labels=labels, X_query=X_query, m=m,
        kappa=kappa, nu=nu, triu_diag=triu_diag, triu_lower=triu_lower,
        n_classes=n_classes))
    res = run_bass_kernel_spmd(nc, in_maps, list(range(N_CORES)))
    return np.concatenate([res.results[i]["out"] for i in range(N_CORES)],
                          axis=0)
